# revision 4
# baseline (speedup 1.0000x reference)
"""Single-head attention (B=8, N=2048, E=1024) on 8 TRN2 NeuronCores.

Sharding: data-parallel over batch — core i computes batch element i fully.

Algebraic fusion (the big PE saving vs the naive graph): q and k only ever
appear through scores = (x Wq^T + bq)(x Wk^T + bk)^T.  Expanding and using
softmax's row-shift invariance (terms constant over the key axis j drop):
    scores ~ x C x^T + 1 w^T,   C = Wq^T Wk  (weight-only, folded on host),
                                w = x (Wk^T bq)  (zero for zero bias).
So the device never computes k at all — the scores matmul contracts
A = x C against xT, which is already resident in SBUF as the projection
input.  This removes the whole k-projection (1/4 of projection FLOPs,
~55us of PE per core).  w rides the (previously unused) bias slot of the
exp activation.

Per-core dataflow (all matmul compute in bf16, f32 PSUM accumulation):
  AT[e,n] = C_lhsT.T @ xT_rhs       (A-projection, e on partitions)
  v[n,e]  = xT_lhsT.T @ WvT_rhs     (natural layout, n on partitions)
  scoresT[j,i] = xT_lhsT.T @ AT_rhs ; expT = exp(scale*scoresT + w[j])
  denom[i] = ones-matmul over j-partitions of DVE-reduced exp sums
  out[i,e] = (expT_lhsT.T @ v_rhs) * (1/denom)
Softmax skips max-subtraction: raw scores stay < ~10, exp is safe in f32
and softmax is shift-invariant.

Schedule notes (from trace analysis):
- Startup: per-DMA issue cost is ~0.7us on the issuing ring, so the first
  CW/x0 k-slices are spread round-robin over rings (scalar/sync/gpsimd)
  and later sections ride single whole-section DMAs. A short junk-MM
  burst covers the ~1us until the first slices land.
- PSUM: pproj(6) + psc(2) = 8 banks live during projection; pnum/pden open
  after pproj closes and only reuse banks whose last use ended much
  earlier, so no first-use alias stalls.
- Output staged as bf16 (host upcasts): halves out-DMA bytes.
"""

import numpy as np
import ml_dtypes

P = 128
E = 1024
N = 2048
KO = E // P      # 8 contraction subtiles
NT = N // P      # 16 row tiles
NCH = N // 512   # 4 chunks of 512
SCALE = 0.03125  # 1/sqrt(1024)

_CACHE = {}


def _build():
    import concourse.bacc as bacc
    import concourse.tile as tile
    import concourse.mybir as mybir

    f32 = mybir.dt.float32
    bf16 = mybir.dt.bfloat16
    AF = mybir.ActivationFunctionType
    Alu = mybir.AluOpType

    nc = bacc.Bacc("TRN2", target_bir_lowering=False, debug=False, num_devices=8)
    xT_d = nc.dram_tensor("xT", [E, N], bf16, kind="ExternalInput")
    CW_d = nc.dram_tensor("CW", [E, E], bf16, kind="ExternalInput")
    WV_d = nc.dram_tensor("WV", [E, E], bf16, kind="ExternalInput")
    wb_d = nc.dram_tensor("w_b", [P, 16], f32, kind="ExternalInput")
    bv_d = nc.dram_tensor("b_v", [P, E], f32, kind="ExternalInput")
    out_d = nc.dram_tensor("out", [N, E], bf16, kind="ExternalOutput")

    xT_r = xT_d.ap().rearrange("(ko p) (c n) -> c p ko n", p=P, n=512)
    CW_r = CW_d.ap().rearrange("(ko p) (s f) -> s p ko f", p=P, f=512)
    WV_r = WV_d.ap().rearrange("(ko p) (s f) -> s p ko f", p=P, f=512)
    out_r = out_d.ap().rearrange("(it p) e -> it p e", p=P)

    with tile.TileContext(nc) as tc:
        with (
            tc.tile_pool(name="const", bufs=1) as const,
            tc.tile_pool(name="qkv", bufs=1) as qkv,
            tc.tile_pool(name="psc", bufs=2, space="PSUM") as psc,
        ):
            wb_t = const.tile([P, 16], f32, tag="wb")
            bv_t = const.tile([P, E], f32, tag="bv")
            ones_t = const.tile([P, 1], bf16, tag="ones")

            # AT split per n-chunk so attention chunk ic only depends on
            # the chunks it reads (finer scheduling deps than one big tile)
            aTc = [
                qkv.tile([P, KO, 512], bf16, tag=f"aT{c}", name=f"aT{c}")
                for c in range(NCH)
            ]
            vt = qkv.tile([P, NT, E], bf16, tag="v")

            # x tiles live for the WHOLE kernel now: the scores matmul uses
            # them directly as lhsT (k == x after the C-fold).
            # Early-consumed chunks are k-sliced (8 x 128KB) so slices land
            # incrementally ahead of the k-outer consumption below; late
            # chunks (x2/x3) are single whole-section DMAs to save issue
            # slots.
            xks = {
                c: [
                    qkv.tile([P, 512], bf16, tag=f"x{c}_{k}", name=f"x{c}_{k}")
                    for k in range(KO)
                ]
                for c in (0, 1)
            }
            xt = {
                c: qkv.tile([P, KO, 512], bf16, tag=f"x{c}", name=f"x{c}")
                for c in (2, 3)
            }

            def x_sl(c, k, fsl):
                if c <= 1:
                    return xks[c][k][:, fsl]
                return xt[c][:, k, fsl]

            with (
                tc.tile_pool(name="pin", bufs=1) as pin,
                tc.tile_pool(name="pproj", bufs=6, space="PSUM") as pproj,
            ):
                # DMA plan. Constraints learned from traces: each engine ring
                # has ~4 HW queues and the 5th+ dma_start BLOCKS the issuing
                # engine until an earlier transfer completes, so the scalar
                # ring gets ONLY CW0 (ACT must stay free to drain PSUM), sync
                # carries the bulk, gpsimd (SWDGE, 8 queues) the rest.
                cwk = {
                    s: [
                        pin.tile([P, 512], bf16, tag=f"cw{s}_{k}",
                                 name=f"cw{s}_{k}")
                        for k in range(KO)
                    ]
                    for s in range(2)
                }
                wvt = {
                    s: pin.tile([P, KO, 512], bf16, tag=f"wv{s}", name=f"wv{s}")
                    for s in range(2)
                }

                # Warmup scratch memset rides gpsimd, which exits the
                # framework preamble ~0.8us before DVE — and is emitted
                # before the DMA issues so it isn't queued behind them.
                scratch = pin.tile([P, 512], bf16, tag="warm_in")
                nc.gpsimd.memset(scratch[:], 0.0)

                # Issue in strict consumption order; startup is HBM-bandwidth
                # bound, so a section pulled early steals bandwidth from the
                # one actually needed. Issue cost is ~0.65us per dma_start on
                # the issuing ring, so the per-ring issue queues are balanced
                # so every k-slice is in flight ~2us before the PE needs it:
                # scalar takes only CW0 (ACT must stay free to drain PSUM),
                # sync takes x0+x1 then the late whole sections, gpsimd
                # (SWDGE) takes CW1 and the small constants.
                for k in range(KO):
                    nc.scalar.dma_start(cwk[0][k][:], CW_r[0][:, k, :])
                for k in range(KO):
                    nc.gpsimd.dma_start(cwk[1][k][:], CW_r[1][:, k, :])
                for k in range(KO):
                    nc.sync.dma_start(xks[0][k][:], xT_r[0][:, k, :])
                for k in range(KO):
                    nc.sync.dma_start(xks[1][k][:], xT_r[1][:, k, :])
                nc.gpsimd.dma_start(bv_t[:], bv_d.ap())
                nc.gpsimd.dma_start(wb_t[:], wb_d.ap())
                nc.sync.dma_start(xt[2][:], xT_r[2])
                nc.sync.dma_start(xt[3][:], xT_r[3])
                nc.sync.dma_start(wvt[0][:], WV_r[0])
                nc.sync.dma_start(wvt[1][:], WV_r[1])

                def cw_sl(s, k, fsl):
                    return cwk[s][k][:, fsl]

                # PE warmup: a short junk burst keeps TensorE busy (and the
                # HAM activity window running) for the ~1us until the first
                # input slices land. Junk writes rotate through the psc tag's
                # banks, whose first real use is much later. Results land
                # in a junk DRAM scratch so DCE keeps the chain.
                nc.vector.memset(ones_t[:], 1.0)
                junk_ps = psc.tile([P, 512], f32, tag="ps_s")
                for _ in range(5):
                    nc.tensor.matmul(
                        junk_ps[:, 0:256], lhsT=scratch[:, :P], rhs=scratch[:, 0:256],
                        start=True, stop=True,
                    )
                junk_sb = pin.tile([P, 1], f32, tag="warm_out")
                nc.vector.tensor_copy(junk_sb[:], junk_ps[:, 0:1])
                junk_d = nc.dram_tensor("warm_scratch", [P, 1], f32, kind="Internal")
                nc.gpsimd.dma_start(junk_d.ap(), junk_sb[:])

                # A projection -> AT [e(part), n], per-chunk tiles.
                # The first two blocks (ch0, et0-3 and et4-7) run k-OUTER
                # across 4 PSUM banks each: every newly-landed CW/x0
                # k-slice feeds 4 matmuls, so the PE consumes startup DMA at
                # 1/4 the usual per-slice rate and never stalls on the input
                # stream while it trickles in.
                for blk in range(2):
                    ps4 = [
                        pproj.tile([P, 512], f32, tag="ps", name=f"ps4_{blk}_{i}")
                        for i in range(4)
                    ]
                    for k in range(KO):
                        for i in range(4):
                            nc.tensor.matmul(
                                ps4[i][:],
                                lhsT=cw_sl(blk, k, slice(i * P, (i + 1) * P)),
                                rhs=x_sl(0, k, slice(0, 512)),
                                start=(k == 0),
                                stop=(k == KO - 1),
                            )
                    for i in range(4):
                        et = blk * 4 + i
                        nc.scalar.activation(
                            aTc[0][:, et, :], ps4[i][:], AF.Copy, scale=1.0
                        )
                for ch in range(1, NCH):
                    for et in range(8):
                        ps = pproj.tile([P, 512], f32, tag="ps")
                        for k in range(KO):
                            nc.tensor.matmul(
                                ps[:],
                                lhsT=cw_sl(et // 4, k, slice((et % 4) * P, (et % 4 + 1) * P)),
                                rhs=x_sl(ch, k, slice(0, 512)),
                                start=(k == 0),
                                stop=(k == KO - 1),
                            )
                        nc.scalar.activation(
                            aTc[ch][:, et, :], ps[:], AF.Copy, scale=1.0
                        )

                # v projection -> v [n(part), e]
                for nt in range(NT):
                    for ch2 in range(2):
                        esl = slice(ch2 * 512, (ch2 + 1) * 512)
                        ps = pproj.tile([P, 512], f32, tag="ps")
                        for k in range(KO):
                            nc.tensor.matmul(
                                ps[:],
                                lhsT=x_sl(nt // 4, k, slice((nt % 4) * P, (nt % 4 + 1) * P)),
                                rhs=wvt[ch2][:, k, 0:512],
                                start=(k == 0),
                                stop=(k == KO - 1),
                            )
                        nc.vector.tensor_tensor(
                            out=vt[:, nt, esl],
                            in0=ps[:],
                            in1=bv_t[:, esl],
                            op=Alu.add,
                        )

            with (
                tc.tile_pool(name="attn", bufs=2) as attn,
                # 4 banks: the PV group k's first MM waits the out-copy of
                # group k-4 (~13.8us of slack) — ACT copy lateness from
                # coarsened scheduler waits never reaches the PE.
                tc.tile_pool(name="pnum", bufs=4, space="PSUM") as pnum,
                tc.tile_pool(name="pden", bufs=2, space="PSUM") as pden,
            ):
                # Software pipeline: scores(ic) is emitted before the
                # denominator + numerator of (ic-1), so the DVE exp-sum
                # reduce of chunk ic-1 overlaps with scores matmuls of ic
                # instead of stalling PE.
                def emit_scores(ic):
                    expT = attn.tile([P, NT, 512], bf16, tag="expT", bufs=3)
                    for jt in range(NT):
                        ps = psc.tile([P, 512], f32, tag="ps_s")
                        for k in range(KO):
                            nc.tensor.matmul(
                                ps[:],
                                lhsT=x_sl(jt // 4, k, slice((jt % 4) * P, (jt % 4 + 1) * P)),
                                rhs=aTc[ic][:, k, :],
                                start=(k == 0),
                                stop=(k == KO - 1),
                            )
                        nc.scalar.activation(
                            expT[:, jt, :], ps[:], AF.Exp,
                            bias=wb_t[:, jt : jt + 1], scale=SCALE,
                        )
                    return expT

                def emit_reduce(expT):
                    # softmax denominators, step 1: sum over the 16 j-tiles
                    # (free-dim strided reduce on DVE). Emitted AFTER the
                    # previous tail so DVE's strict FIFO runs that tail's
                    # reciprocals first.
                    sume = attn.tile([P, 512], f32, tag="sume")
                    nc.vector.reduce_sum(
                        sume[:],
                        expT.rearrange("p j i -> p i j"),
                        axis=mybir.AxisListType.X,
                    )
                    # bf16 copy so the cross-partition denominator matmul is a
                    # cheap bf16 op instead of a double-pass fp32 one. On DVE
                    # (not ACT): it waits on the reduce, and ACT's FIFO must
                    # stay clear for the next chunk's EXPs.
                    sume_bf = attn.tile([P, 512], bf16, tag="sume_bf")
                    nc.vector.tensor_copy(sume_bf[:], sume[:])
                    return sume_bf

                def emit_tail(ic, expT, sume):
                    for isub in range(4):
                        it = ic * 4 + isub
                        # step 2: sum over the remaining 128 j-partitions
                        psd = pden.tile([P, 1], f32, tag="ps_d")
                        nc.tensor.matmul(
                            psd[:],
                            lhsT=sume[:, isub * P : (isub + 1) * P],
                            rhs=ones_t[:],
                            start=True,
                            stop=True,
                        )
                        rden = attn.tile([P, 1], f32, tag="rden", bufs=4)
                        nc.vector.reciprocal(rden[:], psd[:])
                        for ch2 in range(2):
                            esl = slice(ch2 * 512, (ch2 + 1) * 512)
                            ps = pnum.tile([P, 512], f32, tag="ps_n")
                            for jt in range(NT):
                                nc.tensor.matmul(
                                    ps[:],
                                    lhsT=expT[:, jt, isub * P : (isub + 1) * P],
                                    rhs=vt[:, jt, esl],
                                    start=(jt == 0),
                                    stop=(jt == NT - 1),
                                )
                            # division on ScalarE (Copy with per-partition
                            # scale) keeps the DVE free so the pden PSUM slot
                            # recycles without stalling the next denom matmul.
                            # out-DMAs all ride sync: an issue costs ~0.7us of
                            # engine time, and ACT has no slack during the
                            # scores phase; sync is otherwise idle here.
                            osb = attn.tile([P, 512], bf16, tag="osb", bufs=4)
                            nc.scalar.activation(osb[:], ps[:], AF.Copy, scale=rden[:])
                            nc.sync.dma_start(out_r[it][:, esl], osb[:])

                prev = None
                for ic in range(NCH):
                    cur_expT = emit_scores(ic)
                    if prev is not None:
                        emit_tail(ic - 1, *prev)
                    cur_sume = emit_reduce(cur_expT)
                    prev = (cur_expT, cur_sume)
                emit_tail(NCH - 1, *prev)
    nc.compile()
    return nc


def get_nc():
    if "nc" not in _CACHE:
        _CACHE["nc"] = _build()
    return _CACHE["nc"]


def prepare_in_maps(x, W_qkv, b_qkv):
    bf = ml_dtypes.bfloat16
    x = np.asarray(x, dtype=np.float32)
    W = np.asarray(W_qkv, dtype=np.float32)
    b = np.asarray(b_qkv, dtype=np.float32)
    assert x.shape == (8, N, E) and W.shape == (3 * E, E) and b.shape == (3 * E,)
    Wq, Wk, Wv = W[:E], W[E : 2 * E], W[2 * E :]
    bq = b[:E]
    xT = np.ascontiguousarray(np.transpose(x, (0, 2, 1))).astype(bf)  # [8, E, N]
    # Weight-only constant fold: scores = x (Wq^T Wk) x^T (+ j-bias term).
    CW = np.ascontiguousarray(Wq.T @ Wk).astype(bf)  # [e', e]
    WV = np.ascontiguousarray(Wv.T).astype(bf)  # [e, f]
    bv = np.ascontiguousarray(np.broadcast_to(b[2 * E :], (P, E)))  # [P, E]
    # The only bias term softmax doesn't kill: w[j] = x[j] . (Wk^T bq),
    # applied per key-partition via the exp activation's bias slot.
    if np.any(bq):
        w = x @ (Wk.T @ bq)  # [8, N]
        wbs = [
            np.ascontiguousarray(SCALE * w[i].reshape(16, P).T, dtype=np.float32)
            for i in range(8)
        ]
    else:
        wbs = [np.zeros((P, 16), dtype=np.float32)] * 8
    return [
        {"xT": xT[i], "CW": CW, "WV": WV, "w_b": wbs[i], "b_v": bv}
        for i in range(8)
    ]


def kernel(x, W_qkv, b_qkv):
    from concourse.bass_utils import run_bass_kernel_spmd

    nc = get_nc()
    in_maps = prepare_in_maps(x, W_qkv, b_qkv)
    res = run_bass_kernel_spmd(nc, in_maps, core_ids=list(range(8)))
    return np.stack(
        [res.results[i]["out"].astype(np.float32) for i in range(8)], axis=0
    )
